# revision 41
# baseline (speedup 1.0000x reference)
"""Multi-head causal attention on 8 TRN2 NeuronCores — one head per core.

Full inputs in, full output out. Per core (head h):
  [Q^T|K^T] = W_qk^T x^T   (PE, bf16, packed: rows 0-63 Q, 64-127 K, then
                            duplicated into both partition halves)
  S^T = (K^T dup)^T (Q^T dup)  (PE, K=128 duplicated contraction — keeps the
                            HAM clock at 2.4GHz; 2x folded into exp scale)
  P^T = exp(S^T/16)        (ScalarE, 2-bank double-buffered groups)
  O^T[v,i] accum += V'[j,(v|1)]^T P^T[j,i]  (PE; row 64 = sumexp)
  out[i,o] = (O^T dup / sumexp)^T [W_o; W_o]/2  (PE + scaled evac)
Host sums the 8 per-head partial outputs (bf16 partials, f32 accumulate).
"""

import numpy as np
import ml_dtypes

import concourse.bass as bass
import concourse.mybir as mybir
import concourse.tile as tile
from concourse import bacc
from concourse.bass_utils import run_bass_kernel_spmd

BF16 = mybir.dt.bfloat16
F32 = mybir.dt.float32

S = 4096
D_IN = 512
D_K = 64
D_V = 64
D_OUT = 512
H = 8
NJT = S // 128   # 32 key tiles
NPAIR = NJT // 2  # 16 key-tile pairs
NCH = S // 512   # 8 query chunks
NCK = D_IN // 128  # 4 contraction chunks for projections

_CACHE = {}


def _emit(nc, tc, ctx_pools):
    import contextlib

    xT_d = nc.dram_tensor("xT", [D_IN, S], BF16, kind="ExternalInput").ap()
    wqk_d = nc.dram_tensor("wqk", [D_IN, 128], BF16, kind="ExternalInput").ap()
    wv_d = nc.dram_tensor("wv", [D_IN, D_V], BF16, kind="ExternalInput").ap()
    wod_d = nc.dram_tensor("wod", [128, D_OUT], BF16, kind="ExternalInput").ap()
    mask_d = nc.dram_tensor("mask", [128, 128], BF16, kind="ExternalInput").ap()
    iden_d = nc.dram_tensor("iden", [128, 128], BF16, kind="ExternalInput").ap()
    out_d = nc.dram_tensor("out", [S, D_OUT], BF16, kind="ExternalOutput").ap()

    Exp = mybir.ActivationFunctionType.Exp

    with contextlib.ExitStack() as ctx:
        const = ctx.enter_context(tc.tile_pool(name="const", bufs=1))
        persist = ctx.enter_context(tc.tile_pool(name="persist", bufs=1))
        small = ctx.enter_context(tc.tile_pool(name="small", bufs=3))
        outp = ctx.enter_context(tc.tile_pool(name="outp", bufs=3))

        # ---- constants (merged DMAs on the fast HWDGE sync queue, issued
        # before the x^T loads so the first projection starts early) ----
        # wqk = [W_q | W_k] packed [512, 128]: one projection matmul fills
        # Q^T into psum rows 0-63 and K^T into rows 64-127
        wqk_sb = const.tile([128, NCK * 128], BF16)
        wv_sb = const.tile([128, NCK * D_V], BF16)
        wod_sb = const.tile([128, D_OUT], BF16)   # [W_o; W_o] * 0.5
        mask_sb = const.tile([128, 128], BF16)
        iden_sb = const.tile([128, 128], BF16)
        # wqk on sync (the eager projection needs it first); the rest on
        # gpsimd so they don't delay the x^T head loads on sync
        for c in range(NCK):
            rows = slice(c * 128, (c + 1) * 128)
            nc.sync.dma_start(out=wqk_sb[:, c * 128:(c + 1) * 128],
                              in_=wqk_d[rows, :])
            nc.gpsimd.dma_start(out=wv_sb[:, c * D_V:(c + 1) * D_V],
                                in_=wv_d[rows, :])
        nc.gpsimd.dma_start(out=wod_sb, in_=wod_d)
        nc.gpsimd.dma_start(out=mask_sb, in_=mask_d)
        nc.gpsimd.dma_start(out=iden_sb, in_=iden_d)

        # persistent activations, duplicated across partition halves so every
        # matmul contracts over the full 128 partitions (HAM stays at 2.4GHz;
        # the 2x from duplication is folded into the exp scale / W_o):
        #  qt2 = [Q^T; Q^T], kt2 = [K^T; K^T]
        qt2 = persist.tile([128, S], BF16)
        kt2 = persist.tile([128, S], BF16)
        vp = persist.tile([128, NJT * 65], BF16)  # V' tiles [128, 65] per jt

        # warm the ScalarE exp table while the input DMAs run, so the first
        # real exp call skips the ~2.6us ACT_TABLE_LOAD + drain
        warm = small.tile([128, 1], BF16, tag="warm")
        nc.vector.memset(warm, 0.0)
        nc.scalar.activation(warm, warm, Exp, scale=1.0)

        # P^T pair tiles: pt2[t] = [128, 2*W], halves = key tiles 2t, 2t+1,
        # col w <-> query (256t + w), W = S - 256t. Two pools: ptB is created
        # only after the x^T pool closes, so its tiles reuse that space.
        ptA = ctx.enter_context(tc.tile_pool(name="ptA", bufs=1))
        pt_pools = {"A": ptA, "B": None}
        pt2 = {}

        # x^T tiles + V^T (freed once projections drain). The first two
        # s-tiles live in their own SEPARATE tiles (tile-granular dependency:
        # the eager projections must not wait on the big tail DMAs), loaded
        # by one merged DMA on the fast sync queue; tails go on gpsimd.
        xtp_ctx = contextlib.ExitStack()
        xtp = xtp_ctx.enter_context(tc.tile_pool(name="xt", bufs=1))
        vpt = xtp.tile([64, S], BF16)       # V^T (dies with x^T tiles)
        # head loads ride the (idle-at-startup) scalar HWDGE queue, in
        # parallel with the wqk const DMAs on sync
        xt_head = xtp.tile([128, NCK * 1024], BF16)  # [128, c, 0:1024 cols]
        for c in range(NCK):
            nc.scalar.dma_start(
                out=xt_head[:, c * 1024:(c + 1) * 1024],
                in_=xT_d[c * 128:(c + 1) * 128, 0:1024])
        xts = []
        for c in range(NCK):
            xt = xtp.tile([128, S - 1024], BF16, tag=f"xt{c}")
            nc.gpsimd.dma_start(out=xt,
                                in_=xT_d[c * 128:(c + 1) * 128, 1024:S])
            xts.append(xt)

        def xt_slice(c, st):
            if st < 2:
                return xt_head[:, c * 1024 + st * 512:c * 1024 + st * 512 + 512]
            return xts[c][:, (st - 2) * 512:(st - 1) * 512]

        # ---- pending-queue filler machinery (PE work during ScalarE exp) ----
        from collections import deque
        pending = deque()
        xt_left = [0]  # outstanding filler closures that read the x^T tiles

        def drain(n):
            if len(pending) > 48:
                n += 4
            for _ in range(n):
                if not pending:
                    return
                pending.popleft()()

        def xt_unit(go):
            xt_left[0] += 1

            def wrapped():
                go()
                xt_left[0] -= 1
            return wrapped

        def proj_qk(st):
            """One packed QK projection s-tile + the two dup DMA streams."""
            def go():
                sl = bass.ts(st, 512)
                ps = psAcc.tile([128, 512], F32, tag="bank", name=f"pqk{st}")
                for c in range(NCK):
                    nc.tensor.matmul(
                        ps,
                        lhsT=wqk_sb[:, c * 128:(c + 1) * 128],
                        rhs=xt_slice(c, st),
                        start=(c == 0),
                        stop=(c == NCK - 1),
                    )
                nc.vector.tensor_copy(qt2[0:64, sl], ps[0:64, :])
                nc.vector.tensor_copy(kt2[64:128, sl], ps[64:128, :])
                # duplicate into the other partition half via SBUF DMA
                nc.sync.dma_start(out=qt2[64:128, sl], in_=qt2[0:64, sl])
                nc.sync.dma_start(out=kt2[0:64, sl], in_=kt2[64:128, sl])
            return go

        def proj_v(st):
            def go():
                sl = bass.ts(st, 512)
                ps = psAcc.tile([64, 512], F32, tag="bank", name=f"pv{st}")
                for c in range(NCK):
                    nc.tensor.matmul(
                        ps,
                        lhsT=wv_sb[:, c * D_V:(c + 1) * D_V],
                        rhs=xt_slice(c, st),
                        start=(c == 0),
                        stop=(c == NCK - 1),
                    )
                nc.vector.tensor_copy(vpt[:, sl], ps)
            return go

        def vp_transpose(jt):
            def go():
                pst = psAcc.tile([128, 64], BF16, tag="bank", name=f"pst{jt}")
                nc.tensor.transpose(
                    pst,
                    vpt[:, jt * 128:(jt + 1) * 128],
                    iden_sb[0:64, 0:64],
                )
                nc.vector.tensor_copy(vp[:, jt * 65:jt * 65 + 64], pst)
            return go

        accs = {}

        def enqueue_ot(c, j2s):
            if c not in accs:
                accs[c] = psAcc.tile([65, 512], F32, tag="bank",
                                     name=f"acc{c}")
            acc = accs[c]
            jt_last = 4 * c + 3

            def ot_mm(j2):
                def go():
                    lo = max(c * 512, j2 * 128)
                    hi = (c + 1) * 512
                    t = j2 >> 1
                    i0 = 256 * t
                    W = S - i0
                    pv = pt2[t].rearrange("p (h w) -> p h w", h=2)
                    nc.tensor.matmul(
                        acc[:, lo - c * 512:hi - c * 512],
                        lhsT=vp[:, j2 * 65:(j2 + 1) * 65],
                        rhs=pv[:, j2 & 1, lo - i0:hi - i0],
                        start=(j2 == 0),
                        stop=(j2 == jt_last),
                    )
                return go

            for j2 in j2s:
                pending.append(ot_mm(j2))

        def enqueue_fin(c):
            acc = accs[c]

            def evac():
                ot_bf = small.tile([65, 512], BF16, tag="otbf")
                nc.vector.tensor_copy(ot_bf, acc)
                se_bf = small.tile([128, 4], BF16, tag="se_bf")
                for ib in range(4):
                    nc.gpsimd.dma_start(
                        out=se_bf[:, ib:ib + 1],
                        in_=ot_bf[64:65, ib * 128:(ib + 1) * 128],
                    )
                rcols = small.tile([128, 4], F32, tag="rcols")
                nc.vector.reciprocal(rcols, se_bf)
                # duplicate O^T rows into both halves for row-tiled out-proj
                otd = small.tile([128, 512], BF16, tag="otd")
                nc.sync.dma_start(out=otd[0:64, :], in_=ot_bf[0:64, :])
                nc.sync.dma_start(out=otd[64:128, :], in_=ot_bf[0:64, :])

                def out_proj(ib):
                    def go():
                        po0 = psAcc.tile([128, 512], F32, tag="bank",
                                         name=f"po{c}_{ib}")
                        po1 = psAcc.tile([128, 512], F32, tag="bank",
                                         name=f"po{c}_{ib + 1}")
                        # K=128 duplicated contraction (wod carries the 0.5)
                        nc.tensor.matmul(
                            po0,
                            lhsT=otd[:, ib * 128:(ib + 1) * 128],
                            rhs=wod_sb,
                            start=True,
                            stop=True,
                        )
                        nc.tensor.matmul(
                            po1,
                            lhsT=otd[:, (ib + 1) * 128:(ib + 2) * 128],
                            rhs=wod_sb,
                            start=True,
                            stop=True,
                        )
                        ob = outp.tile([128, 1024], BF16, tag="ob")
                        for k, po in ((ib, po0), (ib + 1, po1)):
                            # last chunk: ScalarE is idle once exp is done
                            # (c<7 evacs would block the scalar FIFO ahead
                            # of still-pending exp calls)
                            if c >= 7:
                                nc.scalar.mul(
                                    ob[:, (k - ib) * 512:(k - ib) * 512 + 512],
                                    po, rcols[:, k:k + 1])
                            else:
                                nc.vector.tensor_scalar_mul(
                                    ob[:, (k - ib) * 512:(k - ib) * 512 + 512],
                                    po, rcols[:, k:k + 1])
                        # one DMA for both 128-row blocks (SBUF partition dim
                        # stays first; DRAM side reordered to match)
                        nc.sync.dma_start(
                            out=out_d[c * 512 + ib * 128:
                                      c * 512 + (ib + 2) * 128, :]
                            .rearrange("(b p) f -> p b f", p=128),
                            in_=ob.rearrange("p (b f) -> p b f", b=2),
                        )
                    return go

                pending.append(out_proj(0))
                pending.append(out_proj(2))

            pending.append(evac)

        # ---- main pass: row-tiled S^T pairs + exp, fillers in the gaps ----
        with tc.tile_pool(name="psAcc", bufs=4, space="PSUM") as psAcc, \
             tc.tile_pool(name="psB", bufs=2, space="PSUM") as psB:
            # eager: only s-tile 0 (all pair-0 group-0 needs) runs before the
            # first S^T matmul — more eager projections would sit in the PE
            # FIFO at cold clock ahead of it. The rest drain just-in-time.
            proj_qk(0)()
            # ones column of every V' tile (off the startup critical path;
            # needed only by the first O^T matmul)
            nc.vector.memset(
                vp.rearrange("p (j w) -> p j w", w=65)[:, :, 64], 1.0)
            for st in range(1, NCH):
                pending.append(xt_unit(proj_qk(st)))
            for st in range(NCH):
                pending.append(xt_unit(proj_v(st)))
                for j2 in range(4 * st, 4 * st + 4):
                    pending.append(xt_unit(vp_transpose(j2)))

            for t in range(NPAIR):
                i0 = 256 * t
                W = S - i0
                if t == 5:
                    pt_pools["B"] = ctx.enter_context(
                        tc.tile_pool(name="ptB", bufs=1))
                pool = pt_pools["A"] if t <= 4 else pt_pools["B"]
                pt2[t] = pool.tile([128, 2 * W], BF16, tag=f"pt{t}",
                                   name=f"pt{t}")
                pv = pt2[t].rearrange("p (h w) -> p h w", h=2)
                ngrp = (W + 511) // 512
                for g in range(ngrp):
                    w0 = 512 * g
                    Wg = min(512, W - w0)
                    # 2-bank double-buffered group: half h in its own bank;
                    # next group's matmuls overlap this group's exp
                    ps = psB.tile([128, 1024], F32, tag="psB",
                                  name=f"ps{t}_{g}")
                    for half in range(2):
                        # full-128 duplicated contraction (scores doubled;
                        # folded into the exp scale)
                        jt = 2 * t + half
                        nc.tensor.matmul(
                            ps[:, half * 512:half * 512 + Wg],
                            lhsT=kt2[:, jt * 128:(jt + 1) * 128],
                            rhs=qt2[:, i0 + w0:i0 + w0 + Wg],
                            start=True,
                            stop=True,
                        )
                    nc.scalar.activation(
                        pv[:, :, w0:w0 + Wg],
                        ps.rearrange("p (h w) -> p h w", h=2)[:, :, 0:Wg],
                        Exp,
                        scale=0.0625,  # 1/sqrt(64) / 2 (duplicated halves)
                    )
                    # late pairs have few groups left: drain harder so the
                    # O^T backlog lands before the last exp, not after it
                    drain(3 if t < 12 else 10)
                # causal mask on the two diagonal 128x128 blocks
                nc.vector.tensor_mul(pv[:, 0, 0:128], pv[:, 0, 0:128],
                                     mask_sb)
                nc.vector.tensor_mul(pv[:, 1, 128:256], pv[:, 1, 128:256],
                                     mask_sb)
                # chunk c's O^T accumulation: j2 0..4c-1 enqueued at pair
                # 2c-1, {4c,4c+1} at 2c, {4c+2,4c+3}+finish at 2c+1 — keeps
                # the post-S^T tail short (chunk 7 is nearly done by pair 15)
                if t == 0:
                    enqueue_ot(0, range(0, 2))
                elif t % 2 == 1:
                    c = t // 2
                    enqueue_ot(c, range(2 * t, 2 * t + 2))
                    enqueue_fin(c)
                    if c + 1 < NCH:
                        enqueue_ot(c + 1, range(0, 2 * t + 2))
                else:
                    enqueue_ot(t // 2, range(2 * t, 2 * t + 2))
                if xtp_ctx is not None and (xt_left[0] == 0 or t == 4):
                    # drain any straggler x^T consumers, then free the x^T
                    # tiles before the P^T pool reaches peak size
                    while xt_left[0]:
                        pending.popleft()()
                    xtp_ctx.close()
                    xtp_ctx = None
            while pending:
                drain(8)


def _build():
    if "nc" in _CACHE:
        return _CACHE["nc"]
    nc = bacc.Bacc("TRN2", target_bir_lowering=False, debug=False)
    with tile.TileContext(nc) as tc:
        _emit(nc, tc, None)
    nc.compile()
    _CACHE["nc"] = nc
    return nc


def build_in_maps(x, W_q, W_k, W_v, W_o):
    bf = ml_dtypes.bfloat16
    xT = np.ascontiguousarray(x.reshape(S, D_IN).T).astype(bf)
    mask = np.triu(np.ones((128, 128), np.float32)).astype(bf)
    iden = np.eye(128, dtype=np.float32).astype(bf)
    in_maps = []
    for h in range(H):
        wqk = np.concatenate([W_q[h], W_k[h]], axis=1)  # [512, 128]
        # duplicated-contraction out-proj doubles the product; fold 0.5 here
        wod = np.concatenate([W_o[h], W_o[h]], axis=0) * 0.5  # [128, 512]
        in_maps.append({
            "xT": xT,
            "wqk": np.ascontiguousarray(wqk).astype(bf),
            "wv": np.ascontiguousarray(W_v[h]).astype(bf),
            "wod": np.ascontiguousarray(wod).astype(bf),
            "mask": mask,
            "iden": iden,
        })
    return in_maps


def kernel(x, W_q, W_k, W_v, W_o):
    nc = _build()
    in_maps = build_in_maps(x, W_q, W_k, W_v, W_o)
    res = run_bass_kernel_spmd(nc, in_maps, core_ids=list(range(H)))
    out = np.zeros((S, D_OUT), np.float32)
    for h in range(H):
        out += res.results[h]["out"].astype(np.float32)
    return out[None]
